# revision 1
# baseline (speedup 1.0000x reference)
"""Trainium2 Bass kernel for nn_DistributionLoss_6940667150680 (segment_reduce).

Math: with per-class sums S_c = sum_{i: Y_i=c} w_i and counts n_c,
    L2 = sum_i ||w_i - S_{Y_i}/n_{Y_i}||^2 = sum_i ||w_i||^2 - sum_c ||S_c||^2/n_c
so a single streaming pass over w1 (512 MB) suffices. Data-parallel over 8
NeuronCores (125k rows each). Per core:
  - PE: segment sums S[d, c] += w_tile^T @ onehot(Y_tile), fp16 inputs,
    fp32 PSUM accumulation ([128, 500] x 2 banks; split into an A/B pair so
    A's evacuation overlaps the final tiles' matmuls).
  - DVE: builds each [128, 1000] fp16 one-hot tile with a single
    tensor_scalar(is_equal) against a per-partition Y scalar (4x mode).
  - ACT: casts w f32->fp16 and accumulates sum(w^2) via Square+accum_out.
The tiny cross-core combine (sum of 8 [128,1000] partials, bincount of Y,
final scalar) happens on host in float64.
"""

import numpy as np
from contextlib import ExitStack

import concourse.bass as bass
import concourse.tile as tile
from concourse import mybir
from concourse.bass_utils import run_bass_kernel_spmd

N_CORES = 8
D = 128
C = 1000
P = 128          # partitions / rows per tile
SUPER = 8        # row-tiles per super-tile (one DMA + one cast + one square)


def build_program(supers: int):
    """Build the per-core Bass program. Rows processed = supers*8*128 + 128
    (the final "tail" tile comes from a separate, host-padded input)."""
    f32, f16 = mybir.dt.float32, mybir.dt.float16
    tiles = supers * SUPER + 1
    main_rows = supers * SUPER * P

    nc = bass.Bass()
    w_in = nc.dram_tensor("w", [main_rows, D], f32, kind="ExternalInput")
    wtail_in = nc.dram_tensor("wtail", [P, D], f32, kind="ExternalInput")
    yT_in = nc.dram_tensor("yT", [P, tiles], f32, kind="ExternalInput")
    iota_in = nc.dram_tensor("iota", [P, C], f16, kind="ExternalInput")
    s_out = nc.dram_tensor("s_out", [D, C], f32, kind="ExternalOutput")
    s_outA = nc.dram_tensor("s_outA", [D, C], f32, kind="ExternalOutput")
    sq_out = nc.dram_tensor("sq_out", [P, 1], f32, kind="ExternalOutput")

    # [s][p][g][d]: row index = (s*8 + g)*128 + p
    w_view = w_in.rearrange("(s g p) d -> s p g d", p=P, g=SUPER)

    WBUF = 3   # ring depth for w f32 / fp16 tiles (per super-tile)
    OBUF = 8   # ring depth for one-hot tiles (per row tile)

    def dep(frm, to, why):
        tile.add_dep_helper(
            getattr(frm, "ins", frm), getattr(to, "ins", to), reason=why
        )

    def demote(inst, dep_insts):
        """Move provably-redundant sync deps to nosync (ordering only).
        Used for same-engine WAW/WAR (in-order engines) and for deps that
        are transitively covered by another emitted wait; the TS/AC/DMA ISA
        structs only hold one sync wait each."""
        inst = getattr(inst, "ins", inst)
        drop = {getattr(d, "ins", d).name for d in dep_insts}
        syncs = inst.take_sync_dependencies()
        nosyncs = inst.take_nosync_dependencies()
        for name in drop & set(syncs):
            syncs.discard(name)
            nosyncs.add(name)
        inst.set_sync_dependencies(syncs)
        inst.set_nosync_dependencies(nosyncs)

    with tile.TileContext(nc) as tc, ExitStack() as ctx:
        const = ctx.enter_context(tc.tile_pool(name="const", bufs=1))
        psum = ctx.enter_context(tc.tile_pool(name="psum", bufs=1, space="PSUM"))

        iota_sb = const.tile([P, C], f16)
        nc.sync.dma_start(out=iota_sb, in_=iota_in[:, :])
        yT_sb = const.tile([P, tiles], f32)
        yhead = min(128, tiles)
        nc.sync.dma_start(out=yT_sb[:, 0:yhead], in_=yT_in[:, 0:yhead])
        sq_cols = const.tile([P, supers + 2], f32)

        out_sb = const.tile([D, C], f32, name="out_sb")
        out_sbA = const.tile([D, C], f32, name="out_sbA")

        def evac_a():
            ca1 = nc.vector.tensor_copy(out_sbA[:, 0:500], psum_loA)
            ca2 = nc.vector.tensor_copy(out_sbA[:, 500:C], psum_hiA)
            spnA = nc.sync.nop(nofuse=True, hint="spoutA")
            dep(spnA, ca1, "A ready")
            dep(spnA, ca2, "A ready")
            dmaA = nc.sync.dma_start(out=s_outA[:, :], in_=out_sbA)
            dep(dmaA, spnA, "after WAR nop")
            demote(dmaA, [spnA, ca1, ca2] + list(dmas.values()))
        sq_red_sb = const.tile([P, 1], f32, name="sq_red_sb")
        touch = const.tile([1, 8], f32, name="touch")
        # Touch the const tiles on DVE once so no compute op ever needs more
        # than one DMA wait.
        t0 = nc.vector.tensor_copy(touch[0:1, 0:2], iota_sb[0:1, 0:2])
        t1 = nc.vector.tensor_copy(touch[0:1, 2:4], yT_sb[0:1, 0:2])
        touch_writers = [t0, t1]

        # Explicit ring buffers (manual multi-buffering over subtile deps).
        w_ring = const.tile([P, WBUF, SUPER, D], f32, name="w_ring")
        w16_ring = const.tile([P, WBUF, SUPER, D], f16, name="w16_ring")
        oh_ring = const.tile([P, OBUF, C], f16, name="oh_ring")
        sqd_ring = const.tile([P, 2, SUPER * D], f16, name="sqd_ring")

        psum_loA = psum.tile([D, 500], f32)
        psum_hiA = psum.tile([D, 500], f32)
        psum_loB = psum.tile([D, 500], f32)
        psum_hiB = psum.tile([D, 500], f32)
        TSPLIT = max(0, (supers - 10) * SUPER)

        mms = {}      # t -> (mm_lo, mm_hi)
        tss = {}      # t -> tensor_scalar inst
        casts = {}    # s -> ACT cast inst
        claims = {}   # s -> ACT claim inst
        squares = {}  # s -> ACT square inst
        dmas = {}     # s -> w-load DMA inst

        def do_row_tile(w16_slice, cast, t, start, stop):
            oh = oh_ring[:, t % OBUF, :]
            ts = nc.vector.tensor_scalar(
                oh,
                iota_sb,
                yT_sb[:, t : t + 1],
                None,
                mybir.AluOpType.is_equal,
            )
            if t >= OBUF:
                # WAW vs tss[t-OBUF] is transitively covered by this op's own
                # PE wait (the t-OBUF matmuls waited on that tensor_scalar).
                demote(ts, [tss[t - OBUF]])
            tss[t] = ts
            if t < TSPLIT:
                p_lo, p_hi = psum_loA, psum_hiA
                start = start or t == 0
                stop = t == TSPLIT - 1
            else:
                p_lo, p_hi = psum_loB, psum_hiB
                start = t == TSPLIT
                stop = stop
            mm_lo = nc.tensor.matmul(
                p_lo, lhsT=w16_slice, rhs=oh[:, 0:500], start=start, stop=stop
            )
            mm_hi = nc.tensor.matmul(
                p_hi, lhsT=w16_slice, rhs=oh[:, 500:C], start=start, stop=stop
            )
            demote(mm_lo, [cast])
            demote(mm_hi, [cast])
            mms[t] = (mm_lo, mm_hi)
            if t == TSPLIT - 1:
                evac_a()

        def do_super(s, w_src, n_tiles, tile_base, start_first, stop_last):
            j = s % WBUF
            wt = w_ring[:, j, 0:n_tiles, :]
            spnop = None
            if s >= WBUF:
                # DMA instructions hold a single sync wait, which the HWDGE
                # lane-reuse protocol needs; carry the WAR wait on an SP nop
                # instead (the SP queue is FIFO, so the nop's hardware wait
                # also protects the DMA behind it).
                spnop = nc.sync.nop(nofuse=True, hint=f"spc{s}")
                dep(spnop, casts[s - WBUF], "w ring reader done")
            dma = nc.sync.dma_start(out=wt, in_=w_src)
            # DMA-vs-DMA WAW is ordered by the HWDGE ring (FIFO per issuing
            # engine); no semaphore needed.
            demote(dma, list(dmas.values()))
            if spnop is not None:
                dep(dma, spnop, "after WAR nop")
                demote(dma, [spnop] + list(casts.values()) + list(claims.values())
                       + list(squares.values()))
            dmas[s] = dma
            w16 = w16_ring[:, j, 0:n_tiles, :]
            wt_flat = wt.rearrange("p g d -> p (g d)")
            w16_flat = w16.rearrange("p g d -> p (g d)")
            if s >= WBUF:
                # tiny ACT claim write into the fp16 slot carries the PE WAR
                # wait (matmuls of s-WBUF still reading it); the big cast
                # behind it then needs only its DMA wait.
                claim = nc.scalar.activation(
                    w16_ring[0:1, j, 0, 0:2],
                    touch[0:1, 0:2],
                    mybir.ActivationFunctionType.Copy,
                )
                demote(claim, list(casts.values()) + list(claims.values())
                       + list(squares.values()) + list(tss.values()) + touch_writers)
                claims[s] = claim
            cast = nc.scalar.activation(
                w16_flat, wt_flat, mybir.ActivationFunctionType.Copy
            )
            if s >= WBUF:
                demote(cast, [m for pr in mms.values() for m in pr])
                demote(cast, list(casts.values()) + list(claims.values())
                       + list(squares.values()))
                demote(cast, touch_writers)
            casts[s] = cast
            # tiny DVE read of the casted tile: the is_equal/matmul chain then
            # transitively sees the cast without a second wait on the matmul
            tch = nc.vector.tensor_copy(touch[0:1, 6:8], w16_ring[0:1, j, 0, 0:2])
            demote(tch, touch_writers)
            touch_writers.append(tch)
            sqd = sqd_ring[:, s % 2, 0 : n_tiles * D]
            sq = nc.scalar.activation(
                sqd,
                w16_flat,
                mybir.ActivationFunctionType.Square,
                accum_out=sq_cols[:, s : s + 1],
            )
            # same-engine RAW on cast / WAW on sqd: in-order engine, no sem
            demote(sq, [cast] + list(casts.values()) + list(claims.values())
                   + list(squares.values()))
            squares[s] = sq
            for g in range(n_tiles):
                t = tile_base + g
                do_row_tile(
                    w16_ring[:, j, g, :],
                    cast,
                    t,
                    start=(start_first and g == 0),
                    stop=(stop_last and g == n_tiles - 1),
                )

        w0 = w_view[0]  # [p, g, d]
        do_super(0, w0[:, 0:1, :], 1, 0, True, False)
        do_super(1, w0[:, 1:SUPER, :], SUPER - 1, 1, False, False)
        if yhead < tiles:
            # rest of the yT scalars; a DVE touch re-covers the DMA wait so
            # later is_equal ops still carry only their PE wait
            dma_y2 = nc.sync.dma_start(out=yT_sb[:, yhead:tiles], in_=yT_in[:, yhead:tiles])
            demote(dma_y2, list(dmas.values()))
            t1b = nc.vector.tensor_copy(touch[0:1, 2:4], yT_sb[0:1, yhead : yhead + 2])
            demote(t1b, touch_writers)
            touch_writers.append(t1b)
        for s in range(1, supers):
            do_super(s + 1, w_view[s], SUPER, s * SUPER, False, False)
        # tail tile (host-padded to 128 rows)
        do_super(supers + 1, wtail_in[:, :].rearrange("(g p) d -> p g d", g=1),
                 1, supers * SUPER, False, True)

        # evacuate PSUM B -> SBUF -> DRAM (A was evacuated mid-stream)
        cp1 = nc.vector.tensor_copy(out_sb[:, 0:500], psum_loB)
        cp2 = nc.vector.tensor_copy(out_sb[:, 500:C], psum_hiB)
        spn1 = nc.sync.nop(nofuse=True, hint="spout1")
        dep(spn1, cp1, "s_sb ready")
        dep(spn1, cp2, "s_sb ready")
        out_dma1 = nc.sync.dma_start(out=s_out[:, :], in_=out_sb)
        dep(out_dma1, spn1, "after WAR nop")
        demote(out_dma1, [spn1, cp1, cp2])

        red = nc.vector.reduce_sum(sq_red_sb, sq_cols, axis=mybir.AxisListType.X)
        spn2 = nc.sync.nop(nofuse=True, hint="spout2")
        dep(spn2, red, "sq_red ready")
        out_dma2 = nc.sync.dma_start(out=sq_out[:, :], in_=sq_red_sb)
        dep(out_dma2, spn2, "after WAR nop")
        demote(out_dma2, [spn2, red])

        # Tail sync: cover every proc with single-wait SP nops (the SP queue
        # is FIFO, so the stripped tail drain behind them is safe).
        for tail_dep, why in (
            (mms[tiles - 1][1], "PE done"),
            (squares[supers + 1], "ACT done"),
            (out_dma1, "s_out dma done"),
            (out_dma2, "sq_out dma done"),
        ):
            nop = nc.sync.nop(nofuse=True, hint="tailcover")
            dep(nop, tail_dep, why)

    # The kernel-tail drain waits on every proc; its NOP struct cannot hold
    # that many sync waits and the SP-queue nops above already cover them.
    for blk in nc.m.functions[0].blocks:
        for inst in blk.instructions:
            if not isinstance(inst, mybir.InstDrain):
                continue
            si = inst.sync_info
            if si is None or len(si.on_wait) <= 2:
                continue
            inst.sync_info = mybir.SyncInfo(on_wait=[], on_update=list(si.on_update))

    return nc


def make_in_maps(w1: np.ndarray, Y: np.ndarray, supers: int):
    """Shard row-wise across 8 cores; per-core padded tail tile."""
    n = w1.shape[0]
    rows_per_core = n // N_CORES
    main_rows = supers * SUPER * P
    tail_real = rows_per_core - main_rows
    assert 0 < tail_real <= P, (rows_per_core, main_rows)
    tiles = supers * SUPER + 1

    iota = np.ascontiguousarray(
        np.broadcast_to(np.arange(C, dtype=np.float16), (P, C))
    )
    in_maps = []
    for k in range(N_CORES):
        a = k * rows_per_core
        w_main = w1[a : a + main_rows]  # contiguous view, no copy
        wtail = np.zeros((P, D), dtype=np.float32)
        wtail[:tail_real] = w1[a + main_rows : a + rows_per_core]
        ypad = np.zeros(tiles * P, dtype=np.float32)
        ypad[:rows_per_core] = Y[a : a + rows_per_core].astype(np.float32)
        yT = np.ascontiguousarray(ypad.reshape(tiles, P).T)
        in_maps.append({"w": w_main, "wtail": wtail, "yT": yT, "iota": iota})
    return in_maps


def combine(results, Y, n_total):
    """Host-side unshard: sum partial S/sumsq over cores, final scalar in f64."""
    s_total = np.zeros((D, C), dtype=np.float64)
    totsq = 0.0
    for r in results:
        s_total += r["s_out"].astype(np.float64)
        s_total += r["s_outA"].astype(np.float64)
        totsq += float(r["sq_out"].astype(np.float64).sum())
    counts = np.bincount(Y.astype(np.int64), minlength=C).astype(np.float64)
    corr = float(((s_total * s_total).sum(axis=0) / np.maximum(counts, 1.0)).sum())
    return np.float32((totsq - corr) / n_total)


def run_sharded(w1: np.ndarray, Y: np.ndarray, supers: int, trace: bool = False):
    nc = build_program(supers)
    in_maps = make_in_maps(w1, Y, supers)
    out = run_bass_kernel_spmd(nc, in_maps, list(range(N_CORES)), trace=trace)
    value = combine(out.results, Y, w1.shape[0])
    return value, out


def kernel(w1, Y, num_classes=None):
    w1 = np.ascontiguousarray(np.asarray(w1, dtype=np.float32))
    Y = np.asarray(Y)
    assert w1.shape == (1_000_000, 128) and int(np.asarray(num_classes)) == C
    # 125000 rows/core = 122 super-tiles (124928 rows) + 72-row tail tile
    value, _ = run_sharded(w1, Y, supers=122, trace=False)
    return value



# revision 19
# speedup vs baseline: 3.6006x; 3.6006x over previous
"""Trainium2 Bass kernel for nn_DistributionLoss_6940667150680 (segment_reduce).

Math: with per-class sums S_c = sum_{i: Y_i=c} w_i and counts n_c,
    L2 = sum_i ||w_i - S_{Y_i}/n_{Y_i}||^2 = sum_i ||w_i||^2 - sum_c ||S_c||^2/n_c
so a single streaming pass over w1 suffices.

Sharding strategy (segment-key sharding): the host routes rows by class --
rows are stably sorted by label and each class is padded with zero rows to a
multiple of 128 so that every 128-row tile belongs to exactly one class.  The
padded tile stream is split evenly across the 8 cores.  Each core then only
needs per-TILE column sums (S_c = sum of its tiles' sums, reduced on host),
which turns the segment reduction into a dense streaming reduce:

  - PE: per tile one matmul with a constant selector mask e_i [128, 32]
    (column i all-ones) as the stationary operand: psum accumulates
    e_i^T @ w_tile, i.e. the tile's column sum lands in psum row i and zeros
    elsewhere (M=32, N=128, fp16 in / f32 psum; 32-matmul accumulation group
    per chunk).  No per-tile weight loads of w, no one-hot build on DVE.
  - ACT: Square activation with accum_out on a slice of each chunk.
  - DVE: fused tensor_tensor_reduce (w*w, sum) on the rest, plus tiny
    psum->SBUF evacuations of the per-tile sums.
  - Host: fp16 cast + class-sorted layout (input prep), per-class reduction
    of tile sums, counts via bincount, final scalar in float64.
"""

import numpy as np
from contextlib import ExitStack

import concourse.bass as bass
import concourse.tile as tile
from concourse import mybir
from concourse.bass_utils import run_bass_kernel_spmd

N_CORES = 8
D = 128          # feature dim
P = 128          # partitions / rows per tile
CHUNK = 32       # tiles per DMA chunk (1 MiB fp16)
RING = 4         # w-ring depth in chunks
NPS = 4          # psum tiles (round-robin per chunk)
ACT_COLS = 2560  # ACT's share of each chunk's 4096 columns (squares split)


def build_program(T: int, act_cols: int = ACT_COLS):
    """Per-core program processing T tiles (T % CHUNK == 0)."""
    f32, f16 = mybir.dt.float32, mybir.dt.float16
    assert T % CHUNK == 0
    nch = T // CHUNK
    CF = CHUNK * D           # columns per chunk
    fdA = min(act_cols, CF)
    fdV = CF - fdA

    nc = bass.Bass()
    w_in = nc.dram_tensor("w", [P, T * D], f16, kind="ExternalInput")
    masks_in = nc.dram_tensor("masks", [P, CHUNK * CHUNK], f16, kind="ExternalInput")
    ts_out = nc.dram_tensor("ts_out", [CHUNK, nch * D], f32, kind="ExternalOutput")
    sqa_out = nc.dram_tensor("sqa_out", [P, nch], f32, kind="ExternalOutput")
    sqv_out = nc.dram_tensor("sqv_out", [P, max(nch, 1)], f32, kind="ExternalOutput")

    def dep(frm, to, why):
        tile.add_dep_helper(
            getattr(frm, "ins", frm), getattr(to, "ins", to), reason=why
        )

    def demote(inst, dep_insts):
        """Move provably-redundant sync deps to nosync (ordering only):
        same-engine WAW/WAR (in-order engines) and deps transitively covered
        by another emitted wait (ISA structs hold one sync wait each)."""
        inst = getattr(inst, "ins", inst)
        drop = set()
        for d in dep_insts:
            if d is None:
                continue
            drop.add(getattr(d, "ins", d).name)
        syncs = inst.take_sync_dependencies()
        nosyncs = inst.take_nosync_dependencies()
        for name in drop & set(syncs):
            syncs.discard(name)
            nosyncs.add(name)
        inst.set_sync_dependencies(syncs)
        inst.set_nosync_dependencies(nosyncs)

    # Pin each engine queue to emission order with demoted (nosync) chain
    # edges: the tile scheduler may otherwise reorder within a queue, which
    # breaks every "covered transitively via in-order engine" argument below.
    last_on = {}

    def chain(inst, engine):
        prev = last_on.get(engine)
        if prev is not None:
            dep(inst, prev, "queue order")
            demote(inst, [prev])
        last_on[engine] = inst
        return inst

    with tile.TileContext(nc) as tc, ExitStack() as ctx:
        const = ctx.enter_context(tc.tile_pool(name="const", bufs=1))
        psum = ctx.enter_context(tc.tile_pool(name="psum", bufs=1, space="PSUM"))

        masks_sb = const.tile([P, CHUNK * CHUNK], f16, name="masks_sb")
        dma_masks = nc.sync.dma_start(out=masks_sb, in_=masks_in[:, :])
        w_ring = const.tile([P, RING, CF], f16, name="w_ring")
        sqa_cols = const.tile([P, nch], f32, name="sqa_cols")
        sqv_cols = const.tile([P, max(nch, 1)], f32, name="sqv_cols")
        scrA = const.tile([P, 2, fdA], f16, name="scrA")
        scrV = const.tile([P, 2, max(fdV, 2)], f16, name="scrV")
        out_sb = const.tile([CHUNK, nch * D], f32, name="out_sb")
        touch = const.tile([1, 4], f32, name="touch")

        pst = [psum.tile([CHUNK, D], f32, name=f"pst{k}") for k in range(NPS)]

        # DVE touch of the mask const so the first matmul needs only one wait.
        t_masks = chain(nc.vector.tensor_copy(touch[0:1, 0:1], masks_sb[0:1, 0:1]),
                        "dve")
        touch_writers = [t_masks]

        copies = {}
        mm_last = {}
        acts = {}
        ttrs = {}
        dmas = {}
        readers = {}  # chunk -> every instruction that reads its ring slot

        for c in range(nch):
            j = c % RING
            slot = w_ring[:, j, :]
            # Ring-slot WAR: carry waits on SP nops (the chain edges keep the
            # DMA behind them in the SP queue, so their hardware waits also
            # protect it).
            if c >= RING:
                n1 = chain(nc.sync.nop(nofuse=True, hint=f"war{c}a"), "sp")
                dep(n1, acts[c - RING], "act reader done")
                if (c - RING) in ttrs:
                    n1b = chain(nc.sync.nop(nofuse=True, hint=f"war{c}c"), "sp")
                    dep(n1b, ttrs[c - RING], "ttr reader done")
                n2 = chain(nc.sync.nop(nofuse=True, hint=f"war{c}b"), "sp")
                dep(n2, mm_last[c - RING], "pe reader done")
            dma = chain(
                nc.sync.dma_start(out=slot, in_=w_in[:, c * CF : (c + 1) * CF]),
                "sp",
            )
            # WAR waits live on the nops just above; DMA-vs-DMA WAW is ordered
            # by the HWDGE ring (FIFO per issuing engine).  WAR deps can be
            # re-attached against ANY prior occupant's readers, so demote all.
            demote(dma, list(dmas.values())
                   + [r for k in range(c) for r in readers[k]])
            dmas[c] = dma
            readers[c] = []

            # DVE touch: carries this chunk's DMA wait; the chain edges pin it
            # after copy_{c-NPS} on the in-order DVE queue, so a wait on it
            # also covers the psum-tile WAR for this chunk's matmuls.
            tch = chain(nc.vector.tensor_copy(touch[0:1, 1:2], slot[0:1, 0:1]),
                        "dve")
            demote(tch, touch_writers)  # same-engine WAW on the touch tile
            touch_writers.append(tch)
            readers[c].append(tch)

            # PE: one matmul per tile; e_i^T @ w_tile accumulates the tile's
            # column sum into psum row i (zeros elsewhere).
            pt = pst[c % NPS]
            for i in range(CHUNK):
                mm = chain(
                    nc.tensor.matmul(
                        pt,
                        lhsT=masks_sb[:, i * CHUNK : (i + 1) * CHUNK],
                        rhs=slot[:, i * D : (i + 1) * D],
                        start=(i == 0),
                        stop=(i == CHUNK - 1),
                    ),
                    "pe",
                )
                if i == 0:
                    dep(mm, tch, "chunk + psum ready (transitive)")
                # i == 0: dma/psum-WAR covered via the touch; i > 0: covered
                # transitively (PE chain keeps them behind mm[0]).
                demote(mm, [dma, dma_masks, t_masks, copies.get(c - NPS),
                            mm_last.get(c - NPS)])
                readers[c].append(mm)
            mm_last[c] = mm

            # ACT: squares of the first fdA columns, accumulated per chunk.
            act = chain(
                nc.scalar.activation(
                    scrA[:, c % 2, :],
                    slot[:, 0:fdA],
                    mybir.ActivationFunctionType.Square,
                    accum_out=sqa_cols[:, c : c + 1],
                ),
                "act",
            )
            demote(act, list(acts.values()))  # same-engine WAW on scrA
            acts[c] = act
            readers[c].append(act)

            # DVE: fused square+reduce of the remaining columns.  Carries its
            # own DMA wait (it may not sit right behind the touch).
            if fdV > 0:
                ttr = chain(
                    nc.vector.scalar_tensor_tensor(
                        out=scrV[:, c % 2, :],
                        in0=slot[:, fdA:CF],
                        scalar=1.0,
                        in1=slot[:, fdA:CF],
                        op0=mybir.AluOpType.mult,
                        op1=mybir.AluOpType.mult,
                        accum_out=sqv_cols[:, c : c + 1],
                    ),
                    "dve",
                )
                demote(ttr, list(ttrs.values()))  # scrV WAW same-engine
                ttrs[c] = ttr
                readers[c].append(ttr)

            # DVE: evacuate this chunk's tile sums psum -> SBUF.
            cp = chain(nc.vector.tensor_copy(out_sb[:, c * D : (c + 1) * D], pt),
                       "dve")
            demote(cp, [mm_last[k] for k in range(c)] + list(copies.values()))
            copies[c] = cp

        # Outputs: each DMA waits on the last producer via an SP nop (the
        # producing engines are in-order, so last implies all).
        outs = []
        for name, buf, last in (
            ("ts", ts_out, copies[nch - 1]),
            ("sqa", sqa_out, acts[nch - 1]),
            ("sqv", sqv_out, ttrs.get(nch - 1)),
        ):
            if last is None:
                continue
            spn = chain(nc.sync.nop(nofuse=True, hint=f"out_{name}"), "sp")
            dep(spn, last, f"{name} ready")
            src = {"ts": out_sb, "sqa": sqa_cols, "sqv": sqv_cols}[name]
            od = chain(nc.sync.dma_start(out=buf[:, :], in_=src), "sp")
            dep(od, spn, "after producer nop")
            demote(od, [spn, last] + list(dmas.values()) + outs
                   + list(copies.values()) + list(acts.values())
                   + list(ttrs.values()))
            outs.append(od)

        # Tail sync: cover every proc with single-wait SP nops.
        tails = [mm_last[nch - 1], acts[nch - 1], copies[nch - 1]] + outs
        if (nch - 1) in ttrs:
            tails.append(ttrs[nch - 1])
        for t in tails:
            nop = chain(nc.sync.nop(nofuse=True, hint="tailcover"), "sp")
            dep(nop, t, "tail")

    # The kernel-tail drain waits on every proc; its NOP struct cannot hold
    # that many sync waits and the SP-queue nops above already cover them.
    for blk in nc.m.functions[0].blocks:
        for inst in blk.instructions:
            if not isinstance(inst, mybir.InstDrain):
                continue
            si = inst.sync_info
            if si is None or len(si.on_wait) <= 2:
                continue
            inst.sync_info = mybir.SyncInfo(on_wait=[], on_update=list(si.on_update))

    return nc


def prepare_inputs(w1: np.ndarray, Y: np.ndarray, num_classes: int):
    """Class-sorted, per-class tile-padded, per-core partition-major fp16."""
    n = w1.shape[0]
    counts = np.bincount(Y, minlength=num_classes).astype(np.int64)
    tpc_class = (counts + P - 1) // P          # tiles per class
    pad_start = np.zeros(num_classes + 1, dtype=np.int64)
    np.cumsum(tpc_class, out=pad_start[1:])
    tt = int(pad_start[-1])                    # total real tiles
    t_core = -(-tt // N_CORES)                 # ceil
    t_core = -(-t_core // CHUNK) * CHUNK       # round up to chunk
    t_total = t_core * N_CORES

    order = np.argsort(Y, kind="stable")
    y_sorted = Y[order]
    class_start = np.zeros(num_classes, dtype=np.int64)
    class_start[1:] = np.cumsum(counts)[:-1]
    rank = np.arange(n, dtype=np.int64) - class_start[y_sorted]
    dest = pad_start[y_sorted] * P + rank

    w16 = np.zeros((t_total * P, D), dtype=np.float16)
    w16[dest] = w1[order]

    # selector masks: mask_i[r, m] = (m == i), laid out [P, i*CHUNK + m]
    masks = np.ascontiguousarray(
        np.broadcast_to(np.eye(CHUNK, dtype=np.float16).reshape(1, CHUNK * CHUNK),
                        (P, CHUNK * CHUNK))
    )
    in_maps = []
    for k in range(N_CORES):
        blk = w16[k * t_core * P : (k + 1) * t_core * P]
        wk = np.ascontiguousarray(
            blk.reshape(t_core, P, D).transpose(1, 0, 2).reshape(P, t_core * D)
        )
        in_maps.append({"w": wk, "masks": masks})
    return in_maps, t_core, pad_start, counts


def combine(results, t_core, pad_start, counts, n_total):
    """Host-side: tile sums -> class sums -> final scalar, in float64."""
    nch = t_core // CHUNK
    tile_sums = np.concatenate(
        [
            r["ts_out"].astype(np.float64)
            .reshape(CHUNK, nch, D).transpose(1, 0, 2).reshape(t_core, D)
            for r in results
        ],
        axis=0,
    )  # [t_total, D]
    num_classes = len(counts)
    totsq = 0.0
    for r in results:
        totsq += float(r["sqa_out"].astype(np.float64).sum())
        totsq += float(r["sqv_out"].astype(np.float64).sum())
    # per-class sums: classes are tile-aligned runs of tile_sums
    corr = 0.0
    seg = np.add.reduceat(tile_sums[: pad_start[-1]], pad_start[:-1], axis=0) \
        if pad_start[-1] > 0 else np.zeros((num_classes, D))
    # reduceat quirk: empty segments (pad_start[c]==pad_start[c+1]) copy the
    # row at that index instead of 0 -- mask them out via counts.
    nz = counts > 0
    s = seg[nz]
    corr = float(((s * s).sum(axis=1) / counts[nz]).sum())
    return np.float32((totsq - corr) / n_total)


def run_sharded(w1: np.ndarray, Y: np.ndarray, num_classes: int, trace: bool = False):
    w1 = np.ascontiguousarray(np.asarray(w1, dtype=np.float32))
    Y = np.asarray(Y).astype(np.int64)
    in_maps, t_core, pad_start, counts = prepare_inputs(w1, Y, num_classes)
    nc = build_program(t_core)
    out = run_bass_kernel_spmd(nc, in_maps, list(range(N_CORES)), trace=trace)
    value = combine(out.results, t_core, pad_start, counts, w1.shape[0])
    return value, out


def kernel(w1, Y, num_classes=None):
    w1 = np.asarray(w1, dtype=np.float32)
    Y = np.asarray(Y)
    c = int(np.asarray(num_classes)) if num_classes is not None else 1000
    assert w1.ndim == 2 and w1.shape[1] == D
    value, _ = run_sharded(w1, Y, c, trace=False)
    return value


# revision 20
# speedup vs baseline: 4.3423x; 1.2060x over previous
"""Trainium2 Bass kernel for nn_DistributionLoss_6940667150680 (segment_reduce).

Math: with per-class sums S_c = sum_{i: Y_i=c} w_i and counts n_c,
    L2 = sum_i ||w_i - S_{Y_i}/n_{Y_i}||^2 = sum_i ||w_i||^2 - sum_c ||S_c||^2/n_c
so a single streaming pass over w1 suffices.

Sharding strategy (segment-key sharding): the host routes rows by class --
rows are stably sorted by label and each class is padded with zero rows to a
multiple of 128 so that every 128-row tile belongs to exactly one class.  The
padded tile stream is split evenly across the 8 cores.  Each core then only
needs per-TILE column sums (S_c = sum of its tiles' sums, reduced on host),
which turns the segment reduction into a dense streaming reduce:

  - PE: per tile one matmul with a constant selector mask e_i [128, 32]
    (column i all-ones) as the stationary operand: psum accumulates
    e_i^T @ w_tile, i.e. the tile's column sum lands in psum row i and zeros
    elsewhere (M=32, N=128, fp16 in / f32 psum; 32-matmul accumulation group
    per chunk).  No per-tile weight loads of w, no one-hot build on DVE.
  - ACT: Square activation with accum_out on a slice of each chunk.
  - DVE: fused tensor_tensor_reduce (w*w, sum) on the rest, plus tiny
    psum->SBUF evacuations of the per-tile sums.
  - Host: fp16 cast + class-sorted layout (input prep), per-class reduction
    of tile sums, counts via bincount, final scalar in float64.
"""

import ml_dtypes
import numpy as np
from contextlib import ExitStack

import concourse.bass as bass
import concourse.tile as tile
from concourse import mybir
from concourse.bass_utils import run_bass_kernel_spmd

N_CORES = 8
D = 128          # feature dim
P = 128          # partitions / rows per tile
CHUNK = 32       # tiles per DMA chunk
RING = 4         # w-ring depth in chunks
NPS = 4          # psum tiles (round-robin per chunk)
USE_FP8 = True   # stream w as fp8 e4m3 (halves HBM traffic; rel err ~7e-4)
ACT_COLS = 2380 if USE_FP8 else 2560  # ACT's share of each chunk's columns
NP_DT = ml_dtypes.float8_e4m3 if USE_FP8 else np.float16


def build_program(T: int, act_cols: int = ACT_COLS):
    """Per-core program processing T tiles (T % CHUNK == 0)."""
    f32, f16 = mybir.dt.float32, mybir.dt.float16
    fdt = mybir.dt.float8e4 if USE_FP8 else f16
    assert T % CHUNK == 0
    nch = T // CHUNK
    CF = CHUNK * D           # columns per chunk
    fdA = min(act_cols, CF)
    fdV = CF - fdA

    nc = bass.Bass()
    w_in = nc.dram_tensor("w", [P, T * D], fdt, kind="ExternalInput")
    masks_in = nc.dram_tensor("masks", [P, CHUNK * CHUNK], fdt, kind="ExternalInput")
    ts_out = nc.dram_tensor("ts_out", [CHUNK, nch * D], f32, kind="ExternalOutput")
    sqa_out = nc.dram_tensor("sqa_out", [P, nch], f32, kind="ExternalOutput")
    sqv_out = nc.dram_tensor("sqv_out", [P, max(nch, 1)], f32, kind="ExternalOutput")

    def dep(frm, to, why):
        tile.add_dep_helper(
            getattr(frm, "ins", frm), getattr(to, "ins", to), reason=why
        )

    def demote(inst, dep_insts):
        """Move provably-redundant sync deps to nosync (ordering only):
        same-engine WAW/WAR (in-order engines) and deps transitively covered
        by another emitted wait (ISA structs hold one sync wait each)."""
        inst = getattr(inst, "ins", inst)
        drop = set()
        for d in dep_insts:
            if d is None:
                continue
            drop.add(getattr(d, "ins", d).name)
        syncs = inst.take_sync_dependencies()
        nosyncs = inst.take_nosync_dependencies()
        for name in drop & set(syncs):
            syncs.discard(name)
            nosyncs.add(name)
        inst.set_sync_dependencies(syncs)
        inst.set_nosync_dependencies(nosyncs)

    # Pin each engine queue to emission order with demoted (nosync) chain
    # edges: the tile scheduler may otherwise reorder within a queue, which
    # breaks every "covered transitively via in-order engine" argument below.
    last_on = {}

    def chain(inst, engine):
        prev = last_on.get(engine)
        if prev is not None:
            dep(inst, prev, "queue order")
            demote(inst, [prev])
        last_on[engine] = inst
        return inst

    with tile.TileContext(nc) as tc, ExitStack() as ctx:
        const = ctx.enter_context(tc.tile_pool(name="const", bufs=1))
        psum = ctx.enter_context(tc.tile_pool(name="psum", bufs=1, space="PSUM"))

        masks_sb = const.tile([P, CHUNK * CHUNK], fdt, name="masks_sb")
        dma_masks = nc.sync.dma_start(out=masks_sb, in_=masks_in[:, :])
        w_ring = const.tile([P, RING, CF], fdt, name="w_ring")
        sqa_cols = const.tile([P, nch], f32, name="sqa_cols")
        sqv_cols = const.tile([P, max(nch, 1)], f32, name="sqv_cols")
        scrA = const.tile([P, 2, fdA], f16, name="scrA")  # f16 out: accum runs fp32 internally
        scrV = const.tile([P, 2, max(fdV, 2)], f16, name="scrV")
        out_sb = const.tile([CHUNK, nch * D], f32, name="out_sb")
        touch = const.tile([1, 4], f32, name="touch")

        pst = [psum.tile([CHUNK, D], f32, name=f"pst{k}") for k in range(NPS)]

        # DVE touch of the mask const so the first matmul needs only one wait.
        t_masks = chain(nc.vector.tensor_copy(touch[0:1, 0:1], masks_sb[0:1, 0:1]),
                        "dve")
        touch_writers = [t_masks]

        copies = {}
        mm_last = {}
        acts = {}
        ttrs = {}
        dmas = {}
        readers = {}  # chunk -> every instruction that reads its ring slot

        for c in range(nch):
            j = c % RING
            slot = w_ring[:, j, :]
            # Ring-slot WAR: carry waits on SP nops (the chain edges keep the
            # DMA behind them in the SP queue, so their hardware waits also
            # protect it).
            if c >= RING:
                n1 = chain(nc.sync.nop(nofuse=True, hint=f"war{c}a"), "sp")
                dep(n1, acts[c - RING], "act reader done")
                if (c - RING) in ttrs:
                    n1b = chain(nc.sync.nop(nofuse=True, hint=f"war{c}c"), "sp")
                    dep(n1b, ttrs[c - RING], "ttr reader done")
                n2 = chain(nc.sync.nop(nofuse=True, hint=f"war{c}b"), "sp")
                dep(n2, mm_last[c - RING], "pe reader done")
            dma = chain(
                nc.sync.dma_start(out=slot, in_=w_in[:, c * CF : (c + 1) * CF]),
                "sp",
            )
            # WAR waits live on the nops just above; DMA-vs-DMA WAW is ordered
            # by the HWDGE ring (FIFO per issuing engine).  WAR deps can be
            # re-attached against ANY prior occupant's readers, so demote all.
            demote(dma, list(dmas.values())
                   + [r for k in range(c) for r in readers[k]])
            dmas[c] = dma
            readers[c] = []

            # DVE touch: carries this chunk's DMA wait; the chain edges pin it
            # after copy_{c-NPS} on the in-order DVE queue, so a wait on it
            # also covers the psum-tile WAR for this chunk's matmuls.
            tch = chain(nc.vector.tensor_copy(touch[0:1, 1:2], slot[0:1, 0:1]),
                        "dve")
            demote(tch, touch_writers)  # same-engine WAW on the touch tile
            touch_writers.append(tch)
            readers[c].append(tch)

            # PE: one matmul per tile; e_i^T @ w_tile accumulates the tile's
            # column sum into psum row i (zeros elsewhere).
            pt = pst[c % NPS]
            for i in range(CHUNK):
                mm = chain(
                    nc.tensor.matmul(
                        pt,
                        lhsT=masks_sb[:, i * CHUNK : (i + 1) * CHUNK],
                        rhs=slot[:, i * D : (i + 1) * D],
                        start=(i == 0),
                        stop=(i == CHUNK - 1),
                    ),
                    "pe",
                )
                if i == 0:
                    dep(mm, tch, "chunk + psum ready (transitive)")
                # i == 0: dma/psum-WAR covered via the touch; i > 0: covered
                # transitively (PE chain keeps them behind mm[0]).
                demote(mm, [dma, dma_masks, t_masks, copies.get(c - NPS),
                            mm_last.get(c - NPS)])
                readers[c].append(mm)
            mm_last[c] = mm

            # ACT: squares of the first fdA columns, accumulated per chunk.
            act = chain(
                nc.scalar.activation(
                    scrA[:, c % 2, :],
                    slot[:, 0:fdA],
                    mybir.ActivationFunctionType.Square,
                    accum_out=sqa_cols[:, c : c + 1],
                ),
                "act",
            )
            demote(act, list(acts.values()))  # same-engine WAW on scrA
            acts[c] = act
            readers[c].append(act)

            # DVE: fused square+reduce of the remaining columns.  Carries its
            # own DMA wait (it may not sit right behind the touch).
            if fdV > 0:
                ttr = chain(
                    nc.vector.scalar_tensor_tensor(
                        out=scrV[:, c % 2, :],
                        in0=slot[:, fdA:CF],
                        scalar=1.0,
                        in1=slot[:, fdA:CF],
                        op0=mybir.AluOpType.mult,
                        op1=mybir.AluOpType.mult,
                        accum_out=sqv_cols[:, c : c + 1],
                    ),
                    "dve",
                )
                demote(ttr, list(ttrs.values()))  # scrV WAW same-engine
                ttrs[c] = ttr
                readers[c].append(ttr)

            # DVE: evacuate this chunk's tile sums psum -> SBUF.
            cp = chain(nc.vector.tensor_copy(out_sb[:, c * D : (c + 1) * D], pt),
                       "dve")
            demote(cp, [mm_last[k] for k in range(c)] + list(copies.values()))
            copies[c] = cp

        # Outputs: each DMA waits on the last producer via an SP nop (the
        # producing engines are in-order, so last implies all).
        outs = []
        for name, buf, last in (
            ("ts", ts_out, copies[nch - 1]),
            ("sqa", sqa_out, acts[nch - 1]),
            ("sqv", sqv_out, ttrs.get(nch - 1)),
        ):
            if last is None:
                continue
            spn = chain(nc.sync.nop(nofuse=True, hint=f"out_{name}"), "sp")
            dep(spn, last, f"{name} ready")
            src = {"ts": out_sb, "sqa": sqa_cols, "sqv": sqv_cols}[name]
            od = chain(nc.sync.dma_start(out=buf[:, :], in_=src), "sp")
            dep(od, spn, "after producer nop")
            demote(od, [spn, last] + list(dmas.values()) + outs
                   + list(copies.values()) + list(acts.values())
                   + list(ttrs.values()))
            outs.append(od)

        # Tail sync: cover every proc with single-wait SP nops.
        tails = [mm_last[nch - 1], acts[nch - 1], copies[nch - 1]] + outs
        if (nch - 1) in ttrs:
            tails.append(ttrs[nch - 1])
        for t in tails:
            nop = chain(nc.sync.nop(nofuse=True, hint="tailcover"), "sp")
            dep(nop, t, "tail")

    # The kernel-tail drain waits on every proc; its NOP struct cannot hold
    # that many sync waits and the SP-queue nops above already cover them.
    for blk in nc.m.functions[0].blocks:
        for inst in blk.instructions:
            if not isinstance(inst, mybir.InstDrain):
                continue
            si = inst.sync_info
            if si is None or len(si.on_wait) <= 2:
                continue
            inst.sync_info = mybir.SyncInfo(on_wait=[], on_update=list(si.on_update))

    return nc


def prepare_inputs(w1: np.ndarray, Y: np.ndarray, num_classes: int):
    """Class-sorted, per-class tile-padded, per-core partition-major fp16."""
    n = w1.shape[0]
    counts = np.bincount(Y, minlength=num_classes).astype(np.int64)
    tpc_class = (counts + P - 1) // P          # tiles per class
    pad_start = np.zeros(num_classes + 1, dtype=np.int64)
    np.cumsum(tpc_class, out=pad_start[1:])
    tt = int(pad_start[-1])                    # total real tiles
    t_core = -(-tt // N_CORES)                 # ceil
    t_core = -(-t_core // CHUNK) * CHUNK       # round up to chunk
    t_total = t_core * N_CORES

    order = np.argsort(Y, kind="stable")
    y_sorted = Y[order]
    class_start = np.zeros(num_classes, dtype=np.int64)
    class_start[1:] = np.cumsum(counts)[:-1]
    rank = np.arange(n, dtype=np.int64) - class_start[y_sorted]
    dest = pad_start[y_sorted] * P + rank

    w16 = np.zeros((t_total * P, D), dtype=NP_DT)
    w16[dest] = w1[order].astype(NP_DT)

    # selector masks: mask_i[r, m] = (m == i), laid out [P, i*CHUNK + m]
    masks = np.ascontiguousarray(
        np.broadcast_to(np.eye(CHUNK, dtype=NP_DT).reshape(1, CHUNK * CHUNK),
                        (P, CHUNK * CHUNK))
    )
    in_maps = []
    for k in range(N_CORES):
        blk = w16[k * t_core * P : (k + 1) * t_core * P]
        wk = np.ascontiguousarray(
            blk.reshape(t_core, P, D).transpose(1, 0, 2).reshape(P, t_core * D)
        )
        in_maps.append({"w": wk, "masks": masks})
    return in_maps, t_core, pad_start, counts


def combine(results, t_core, pad_start, counts, n_total):
    """Host-side: tile sums -> class sums -> final scalar, in float64."""
    nch = t_core // CHUNK
    tile_sums = np.concatenate(
        [
            r["ts_out"].astype(np.float64)
            .reshape(CHUNK, nch, D).transpose(1, 0, 2).reshape(t_core, D)
            for r in results
        ],
        axis=0,
    )  # [t_total, D]
    num_classes = len(counts)
    totsq = 0.0
    for r in results:
        totsq += float(r["sqa_out"].astype(np.float64).sum())
        totsq += float(r["sqv_out"].astype(np.float64).sum())
    # per-class sums: classes are tile-aligned runs of tile_sums
    corr = 0.0
    seg = np.add.reduceat(tile_sums[: pad_start[-1]], pad_start[:-1], axis=0) \
        if pad_start[-1] > 0 else np.zeros((num_classes, D))
    # reduceat quirk: empty segments (pad_start[c]==pad_start[c+1]) copy the
    # row at that index instead of 0 -- mask them out via counts.
    nz = counts > 0
    s = seg[nz]
    corr = float(((s * s).sum(axis=1) / counts[nz]).sum())
    return np.float32((totsq - corr) / n_total)


def run_sharded(w1: np.ndarray, Y: np.ndarray, num_classes: int, trace: bool = False):
    w1 = np.ascontiguousarray(np.asarray(w1, dtype=np.float32))
    Y = np.asarray(Y).astype(np.int64)
    in_maps, t_core, pad_start, counts = prepare_inputs(w1, Y, num_classes)
    nc = build_program(t_core)
    out = run_bass_kernel_spmd(nc, in_maps, list(range(N_CORES)), trace=trace)
    value = combine(out.results, t_core, pad_start, counts, w1.shape[0])
    return value, out


def kernel(w1, Y, num_classes=None):
    w1 = np.asarray(w1, dtype=np.float32)
    Y = np.asarray(Y)
    c = int(np.asarray(num_classes)) if num_classes is not None else 1000
    assert w1.ndim == 2 and w1.shape[1] == D
    value, _ = run_sharded(w1, Y, c, trace=False)
    return value
